# revision 5
# baseline (speedup 1.0000x reference)
"""ChannelMHSA on Trainium2 (Bass/Tile), data-parallel over batch on 8 cores.

Reference computation (per batch b of x [N, C]):
    qkv  = x @ w_qkv                      # [N, 3C], columns ordered (s, h, d)
    q, k, v per head h: [N, D]
    z_h  = k_h^T @ v_h / sqrt(D)          # [D, D]
    A_h  = softmax(z_h, axis=-1)
    T_h  = A_h @ q_h^T                    # [D, N]
    out[n, h*D+d] = T_h[d, n]
    y    = out @ w_out                    # [N, C]

b_qkv / b_out are all-zero by construction (see input spec) and are ignored.

Kernel layout choices per core (BS=4 batches):
  - xT [C, N] built by PE transposes (6x8 [128,128] blocks per batch).
  - kv = x @ w_qkv[:, C:3C] computed N-major (lhsT = xT chunks).
  - qT = w_q^T @ x^T computed C-major directly (lhsT = w_q chunks,
    rhs = xT chunks), so q never needs a separate transpose.
  - z per head with rhs packed 4 heads wide (free=256) for PE efficiency.
  - A^T placed into a block-diagonal [128,128] lhsT per head pair so
    T for two heads is one K=128 matmul per 512 output columns.
  - y = out @ w_out with lhsT = outT chunks.
"""

import os
import sys
from contextlib import ExitStack

import numpy as np

for _p in ("/opt/trn_rl_repo", "/opt/pypackages"):
    if _p not in sys.path:
        sys.path.append(_p)

import concourse.bacc as bacc
import concourse.mybir as mybir
import concourse.tile as tile
from concourse import bass_utils, masks

B, N, C = 32, 1024, 768
H, D = 12, 64
P = 128
NCORES = 8
BS = B // NCORES          # batches per core
KC = C // P               # 6 contraction chunks over C
NM = N // P               # 8 chunks over N
F32 = mybir.dt.float32
F32R = mybir.dt.float32r

# float32r runs the PE at 4x fp32 speed for free-dim >= 256 with slightly
# reduced mantissa precision. Override with BASS_MM_DT=f32 to compare.
MM_DT_NAME = os.environ.get("BASS_MM_DT", "f32r")


def _emit(ctx, tc, mm_dt, x_d, wqkv_d, wo_d, y_d):
    nc = tc.nc

    def mm(ap):
        return ap.bitcast(mm_dt) if mm_dt is not F32 else ap

    const = ctx.enter_context(tc.tile_pool(name="const", bufs=1))
    xin_pool = ctx.enter_context(tc.tile_pool(name="xin", bufs=2))
    big_pool = ctx.enter_context(tc.tile_pool(name="big", bufs=6))
    kv_pool = ctx.enter_context(tc.tile_pool(name="kvp", bufs=8))
    qt_pool = ctx.enter_context(tc.tile_pool(name="qtp", bufs=6))
    y_pool = ctx.enter_context(tc.tile_pool(name="yp", bufs=2))
    sm_pool = ctx.enter_context(tc.tile_pool(name="smp", bufs=4))
    a2_pool = ctx.enter_context(tc.tile_pool(name="a2p", bufs=2))
    psA = ctx.enter_context(tc.tile_pool(name="psA", bufs=3, space="PSUM"))
    psB = ctx.enter_context(tc.tile_pool(name="psB", bufs=3, space="PSUM"))
    psZ = ctx.enter_context(tc.tile_pool(name="psZ", bufs=2, space="PSUM"))

    ident = const.tile([P, P], F32, tag="ident", name="ident")
    masks.make_identity(nc, ident[:])

    wq, wkv, wo = [], [], []
    for p in range(KC):
        t = const.tile([P, C], F32, tag=f"wq{p}", name=f"wq{p}")
        nc.sync.dma_start(t[:], wqkv_d[p * P:(p + 1) * P, 0:C])
        wq.append(t)
        t = const.tile([P, 2 * C], F32, tag=f"wkv{p}", name=f"wkv{p}")
        nc.sync.dma_start(t[:], wqkv_d[p * P:(p + 1) * P, C:3 * C])
        wkv.append(t)
        t = const.tile([P, C], F32, tag=f"wo{p}", name=f"wo{p}")
        nc.sync.dma_start(t[:], wo_d[p * P:(p + 1) * P, :])
        wo.append(t)

    for b in range(BS):
        # ---- Phase A: load x, transpose to xT [C, N] ----
        xT = [big_pool.tile([P, N], F32, tag="bigT", name=f"xT{b}_{p}")
              for p in range(KC)]
        for m in range(NM):
            xin = xin_pool.tile([P, C], F32, tag="xin", name=f"xin{b}_{m}")
            nc.sync.dma_start(xin[:], x_d[b, m * P:(m + 1) * P, :])
            for p in range(KC):
                tp = psA.tile([P, P], F32, tag="tp", name=f"tpx{b}_{m}_{p}",
                              space="PSUM")
                nc.tensor.transpose(tp[:], xin[:, p * P:(p + 1) * P], ident[:])
                nc.vector.tensor_copy(xT[p][:, m * P:(m + 1) * P], tp[:])

        # ---- Phase B1: kv = x @ w_qkv[:, C:3C], N-major ----
        kv = []
        for m in range(NM):
            kvt = kv_pool.tile([P, 2 * C], F32, tag="kv", name=f"kv{b}_{m}")
            kv.append(kvt)
            for f in range(3):
                ps = psB.tile([P, 512], F32, tag="psB", name=f"pskv{b}_{m}_{f}",
                              space="PSUM")
                for p in range(KC):
                    nc.tensor.matmul(
                        ps[:],
                        mm(xT[p][:, m * P:(m + 1) * P]),
                        mm(wkv[p][:, f * 512:(f + 1) * 512]),
                        start=(p == 0), stop=(p == KC - 1))
                nc.vector.tensor_copy(kvt[:, f * 512:(f + 1) * 512], ps[:])

        # ---- Phase B2: qT = w_q^T @ x^T, C-major ----
        qT = []
        for po in range(KC):
            qtt = qt_pool.tile([P, N], F32, tag="qT", name=f"qT{b}_{po}")
            qT.append(qtt)
            for nf in range(2):
                ps = psB.tile([P, 512], F32, tag="psB", name=f"psqt{b}_{po}_{nf}",
                              space="PSUM")
                for p in range(KC):
                    nc.tensor.matmul(
                        ps[:],
                        mm(wq[p][:, po * P:(po + 1) * P]),
                        mm(xT[p][:, nf * 512:(nf + 1) * 512]),
                        start=(p == 0), stop=(p == KC - 1))
                nc.vector.tensor_copy(qtt[:, nf * 512:(nf + 1) * 512], ps[:])

        # ---- Phase C: attention, one head pair (= one qT tile) at a time ----
        outT = [big_pool.tile([P, N], F32, tag="bigT", name=f"outT{b}_{p}")
                for p in range(KC)]
        for pr in range(KC):
            q4 = pr // 2
            a2 = a2_pool.tile([P, P], F32, tag="a2", name=f"a2_{b}_{pr}")
            nc.vector.memset(a2[:], 0.0)
            for j in range(2):
                h = 2 * pr + j
                # z_h = k_h^T @ v_quad (rhs packs 4 heads of v, free=256)
                zps = psZ.tile([D, 256], F32, tag="z", name=f"z{b}_{h}",
                               space="PSUM")
                for m in range(NM):
                    nc.tensor.matmul(
                        zps[:],
                        mm(kv[m][:, h * D:(h + 1) * D]),
                        mm(kv[m][:, C + q4 * 256:C + (q4 + 1) * 256]),
                        start=(m == 0), stop=(m == NM - 1))
                cb = (h % 4) * D
                zsl = zps[:, cb:cb + D]
                # softmax(z / 8) along free dim
                negmax = sm_pool.tile([D, 1], F32, tag="negmax", name=f"nm{b}_{h}")
                nc.vector.reduce_max(negmax[:], zsl, axis=mybir.AxisListType.X,
                                     negate=True)
                nmx = sm_pool.tile([D, 1], F32, tag="nmx", name=f"nmx{b}_{h}")
                nc.scalar.mul(nmx[:], negmax[:], 0.125)
                aex = sm_pool.tile([D, D], F32, tag="aex", name=f"aex{b}_{h}")
                ssum = sm_pool.tile([D, 1], F32, tag="ssum", name=f"ss{b}_{h}")
                nc.scalar.activation(aex[:], zsl,
                                     mybir.ActivationFunctionType.Exp,
                                     bias=nmx[:], scale=0.125,
                                     accum_out=ssum[:])
                rinv = sm_pool.tile([D, 1], F32, tag="rinv", name=f"ri{b}_{h}")
                nc.vector.reciprocal(rinv[:], ssum[:])
                nc.vector.tensor_scalar_mul(aex[:], aex[:], rinv[:])
                # A^T into block-diag slot j of a2. The PE can only write
                # transpose outputs at PSUM partition 0, and compute engines
                # cannot shift partitions, so the odd head goes through a
                # small SBUF->SBUF DMA to land on partitions 64:128.
                tp = psA.tile([P, D], F32, tag="tp", name=f"tpa{b}_{h}",
                              space="PSUM")
                nc.tensor.transpose(tp[0:D, 0:D], aex[:], ident[0:D, 0:D])
                if j == 0:
                    nc.vector.tensor_copy(a2[0:D, 0:D], tp[0:D, 0:D])
                else:
                    at_sb = sm_pool.tile([D, D], F32, tag="at", name=f"at{b}_{h}")
                    nc.vector.tensor_copy(at_sb[:], tp[0:D, 0:D])
                    nc.sync.dma_start(a2[D:2 * D, D:2 * D], at_sb[:])
            # T for both heads of the pair: one K=128 matmul per 512 cols
            for nf in range(2):
                ps = psB.tile([P, 512], F32, tag="psB", name=f"psT{b}_{pr}_{nf}",
                              space="PSUM")
                nc.tensor.matmul(ps[:], mm(a2[:]),
                                 mm(qT[pr][:, nf * 512:(nf + 1) * 512]),
                                 start=True, stop=True)
                nc.vector.tensor_copy(outT[pr][:, nf * 512:(nf + 1) * 512],
                                      ps[:])

        # ---- Phase D: y = out @ w_out ----
        for m in range(NM):
            yt = y_pool.tile([P, C], F32, tag="y", name=f"y{b}_{m}")
            for f in range(2):
                ps = psB.tile([P, 384], F32, tag="psB", name=f"psy{b}_{m}_{f}",
                              space="PSUM")
                for p in range(KC):
                    nc.tensor.matmul(
                        ps[:],
                        mm(outT[p][:, m * P:(m + 1) * P]),
                        mm(wo[p][:, f * 384:(f + 1) * 384]),
                        start=(p == 0), stop=(p == KC - 1))
                nc.vector.tensor_copy(yt[:, f * 384:(f + 1) * 384], ps[:])
            nc.sync.dma_start(y_d[b, m * P:(m + 1) * P, :], yt[:])


_BUILD_CACHE = {}


def build_program(mm_dt_name=MM_DT_NAME):
    if mm_dt_name in _BUILD_CACHE:
        return _BUILD_CACHE[mm_dt_name]
    mm_dt = F32R if mm_dt_name == "f32r" else F32
    nc = bacc.Bacc("TRN2", target_bir_lowering=False, debug=False,
                   num_devices=NCORES)
    x_d = nc.dram_tensor("x", [BS, N, C], F32, kind="ExternalInput").ap()
    wqkv_d = nc.dram_tensor("w_qkv", [C, 3 * C], F32, kind="ExternalInput").ap()
    wo_d = nc.dram_tensor("w_out", [C, C], F32, kind="ExternalInput").ap()
    y_d = nc.dram_tensor("y", [BS, N, C], F32, kind="ExternalOutput").ap()
    with tile.TileContext(nc) as tc:
        with ExitStack() as ctx:
            _emit(ctx, tc, mm_dt, x_d, wqkv_d, wo_d, y_d)
    nc.compile()
    _BUILD_CACHE[mm_dt_name] = nc
    return nc


def make_in_maps(x, w_qkv, w_out):
    x = np.ascontiguousarray(np.asarray(x, dtype=np.float32))
    w_qkv = np.ascontiguousarray(np.asarray(w_qkv, dtype=np.float32))
    w_out = np.ascontiguousarray(np.asarray(w_out, dtype=np.float32))
    return [
        {"x": x[i * BS:(i + 1) * BS], "w_qkv": w_qkv, "w_out": w_out}
        for i in range(NCORES)
    ]


def kernel(x, w_qkv, b_qkv=None, w_out=None, b_out=None, **_unused):
    nc = build_program()
    in_maps = make_in_maps(x, w_qkv, w_out)
    res = bass_utils.run_bass_kernel_spmd(nc, in_maps,
                                          core_ids=list(range(NCORES)))
    y = np.concatenate([res.results[i]["y"] for i in range(NCORES)], axis=0)
    return np.asarray(y, dtype=np.float32)


# revision 9
# speedup vs baseline: 3.5385x; 3.5385x over previous
"""ChannelMHSA on Trainium2 (Bass/Tile), data-parallel over batch on 8 cores.

Reference computation (per batch b of x [N, C]):
    qkv  = x @ w_qkv                      # [N, 3C], columns ordered (s, h, d)
    q, k, v per head h: [N, D]
    z_h  = k_h^T @ v_h / sqrt(D)          # [D, D]
    A_h  = softmax(z_h, axis=-1)
    T_h  = A_h @ q_h^T                    # [D, N]
    out[n, h*D+d] = T_h[d, n]
    y    = out @ w_out                    # [N, C]

b_qkv / b_out are all-zero by construction (see input spec) and are ignored.

Kernel layout choices per core (BS=4 batches):
  - xT [C, N] built by PE transposes (6x8 [128,128] blocks per batch).
  - kv = x @ w_qkv[:, C:3C] computed N-major (lhsT = xT chunks).
  - qT = w_q^T @ x^T computed C-major directly (lhsT = w_q chunks,
    rhs = xT chunks), so q never needs a separate transpose.
  - z per head with rhs packed 4 heads wide (free=256) for PE efficiency.
  - A^T placed into a block-diagonal [128,128] lhsT per head pair so
    T for two heads is one K=128 matmul per 512 output columns.
  - y = out @ w_out with lhsT = outT chunks.
"""

import os
import sys
from contextlib import ExitStack

import numpy as np

for _p in ("/opt/trn_rl_repo", "/opt/pypackages"):
    if _p not in sys.path:
        sys.path.append(_p)

import concourse.bacc as bacc
import concourse.mybir as mybir
import concourse.tile as tile
from concourse import bass_utils, masks

B, N, C = 32, 1024, 768
H, D = 12, 64
P = 128
NCORES = 8
BS = B // NCORES          # batches per core
KC = C // P               # 6 contraction chunks over C
NM = N // P               # 8 chunks over N
F32 = mybir.dt.float32
F32R = mybir.dt.float32r

# float32r runs the PE at 4x fp32 speed for free-dim >= 256 with slightly
# reduced mantissa precision. Override with BASS_MM_DT=f32 to compare.
MM_DT_NAME = os.environ.get("BASS_MM_DT", "f32r")


def _emit(ctx, tc, mm_dt, x_d, wqkv_d, wo_d, y_d):
    nc = tc.nc

    mdt = mm_dt          # dtype for tiles consumed by regular matmuls
    def wcast(ap):       # DRAM-side view for weight DMAs
        return ap.bitcast(mdt) if mdt is not F32 else ap

    const = ctx.enter_context(tc.tile_pool(name="const", bufs=1))
    xin_pool = ctx.enter_context(tc.tile_pool(name="xin", bufs=2))
    big_pool = ctx.enter_context(tc.tile_pool(name="big", bufs=6))
    kv_pool = ctx.enter_context(tc.tile_pool(name="kvp", bufs=8))
    qt_pool = ctx.enter_context(tc.tile_pool(name="qtp", bufs=6))
    y_pool = ctx.enter_context(tc.tile_pool(name="yp", bufs=2))
    sm_pool = ctx.enter_context(tc.tile_pool(name="smp", bufs=4))
    psA = ctx.enter_context(tc.tile_pool(name="psA", bufs=3, space="PSUM"))
    psB = ctx.enter_context(tc.tile_pool(name="psB", bufs=3, space="PSUM"))
    psZ = ctx.enter_context(tc.tile_pool(name="psZ", bufs=2, space="PSUM"))

    ident = const.tile([P, P], F32, tag="ident", name="ident")
    masks.make_identity(nc, ident[:])

    # Two persistent block-diag lhsT tiles for the T matmul, zeroed once via
    # a rounding copy (memset cannot produce float32r). Only the diagonal
    # blocks are rewritten afterwards, so the off-diag zeros persist.
    zeros = const.tile([P, P], F32, tag="zeros", name="zeros")
    nc.vector.memset(zeros[:], 0.0)
    a2_tiles = []
    for i in range(2):
        a2t = const.tile([P, P], mdt, tag=f"a2_{i}", name=f"a2_{i}")
        nc.vector.tensor_copy(a2t[:], zeros[:])
        a2_tiles.append(a2t)

    wq, wkv, wo = [], [], []
    for p in range(KC):
        t = const.tile([P, C], mdt, tag=f"wq{p}", name=f"wq{p}")
        nc.sync.dma_start(t[:], wcast(wqkv_d[p * P:(p + 1) * P, 0:C]))
        wq.append(t)
        t = const.tile([P, 2 * C], mdt, tag=f"wkv{p}", name=f"wkv{p}")
        nc.sync.dma_start(t[:], wcast(wqkv_d[p * P:(p + 1) * P, C:3 * C]))
        wkv.append(t)
        t = const.tile([P, C], mdt, tag=f"wo{p}", name=f"wo{p}")
        nc.sync.dma_start(t[:], wcast(wo_d[p * P:(p + 1) * P, :]))
        wo.append(t)

    for b in range(BS):
        # ---- Phase A: load x, transpose to xT [C, N] ----
        xT = [big_pool.tile([P, N], mdt, tag="bigT", name=f"xT{b}_{p}")
              for p in range(KC)]
        for m in range(NM):
            xin = xin_pool.tile([P, C], F32, tag="xin", name=f"xin{b}_{m}")
            nc.sync.dma_start(xin[:], x_d[b, m * P:(m + 1) * P, :])
            for p in range(KC):
                tp = psA.tile([P, P], F32, tag="tp", name=f"tpx{b}_{m}_{p}",
                              space="PSUM")
                nc.tensor.transpose(tp[:], xin[:, p * P:(p + 1) * P], ident[:])
                nc.vector.tensor_copy(xT[p][:, m * P:(m + 1) * P], tp[:])

        # ---- Phase B1: kv = x @ w_qkv[:, C:3C], N-major ----
        kv = []
        for m in range(NM):
            kvt = kv_pool.tile([P, 2 * C], mdt, tag="kv", name=f"kv{b}_{m}")
            kv.append(kvt)
            for f in range(3):
                ps = psB.tile([P, 512], F32, tag="psB", name=f"pskv{b}_{m}_{f}",
                              space="PSUM")
                for p in range(KC):
                    nc.tensor.matmul(
                        ps[:],
                        xT[p][:, m * P:(m + 1) * P],
                        wkv[p][:, f * 512:(f + 1) * 512],
                        start=(p == 0), stop=(p == KC - 1))
                nc.vector.tensor_copy(kvt[:, f * 512:(f + 1) * 512], ps[:])

        # ---- Phase B2: qT = w_q^T @ x^T, C-major ----
        qT = []
        for po in range(KC):
            qtt = qt_pool.tile([P, N], mdt, tag="qT", name=f"qT{b}_{po}")
            qT.append(qtt)
            for nf in range(2):
                ps = psB.tile([P, 512], F32, tag="psB", name=f"psqt{b}_{po}_{nf}",
                              space="PSUM")
                for p in range(KC):
                    nc.tensor.matmul(
                        ps[:],
                        wq[p][:, po * P:(po + 1) * P],
                        xT[p][:, nf * 512:(nf + 1) * 512],
                        start=(p == 0), stop=(p == KC - 1))
                nc.vector.tensor_copy(qtt[:, nf * 512:(nf + 1) * 512], ps[:])

        # ---- Phase C: attention, one head pair (= one qT tile) at a time ----
        outT = [big_pool.tile([P, N], mdt, tag="bigT", name=f"outT{b}_{p}")
                for p in range(KC)]
        for pr in range(KC):
            q4 = pr // 2
            a2 = a2_tiles[pr % 2]
            for j in range(2):
                h = 2 * pr + j
                # z_h = k_h^T @ v_quad (rhs packs 4 heads of v, free=256)
                zps = psZ.tile([D, 256], F32, tag="z", name=f"z{b}_{h}",
                               space="PSUM")
                for m in range(NM):
                    nc.tensor.matmul(
                        zps[:],
                        kv[m][:, h * D:(h + 1) * D],
                        kv[m][:, C + q4 * 256:C + (q4 + 1) * 256],
                        start=(m == 0), stop=(m == NM - 1))
                cb = (h % 4) * D
                zsl = zps[:, cb:cb + D]
                # softmax(z / 8) along free dim
                negmax = sm_pool.tile([D, 1], F32, tag="negmax", name=f"nm{b}_{h}")
                nc.vector.reduce_max(negmax[:], zsl, axis=mybir.AxisListType.X,
                                     negate=True)
                nmx = sm_pool.tile([D, 1], F32, tag="nmx", name=f"nmx{b}_{h}")
                nc.scalar.mul(nmx[:], negmax[:], 0.125)
                aex = sm_pool.tile([D, D], F32, tag="aex", name=f"aex{b}_{h}")
                ssum = sm_pool.tile([D, 1], F32, tag="ssum", name=f"ss{b}_{h}")
                nc.scalar.activation(aex[:], zsl,
                                     mybir.ActivationFunctionType.Exp,
                                     bias=nmx[:], scale=0.125,
                                     accum_out=ssum[:])
                rinv = sm_pool.tile([D, 1], F32, tag="rinv", name=f"ri{b}_{h}")
                nc.vector.reciprocal(rinv[:], ssum[:])
                nc.vector.tensor_scalar_mul(aex[:], aex[:], rinv[:])
                # A^T into block-diag slot j of a2. The PE can only write
                # transpose outputs at PSUM partition 0, and compute engines
                # cannot shift partitions, so the odd head goes through a
                # small SBUF->SBUF DMA to land on partitions 64:128.
                tp = psA.tile([P, D], F32, tag="tp", name=f"tpa{b}_{h}",
                              space="PSUM")
                nc.tensor.transpose(tp[0:D, 0:D], aex[:], ident[0:D, 0:D])
                if j == 0:
                    nc.vector.tensor_copy(a2[0:D, 0:D], tp[0:D, 0:D])
                else:
                    at_sb = sm_pool.tile([D, D], mdt, tag="at", name=f"at{b}_{h}")
                    nc.vector.tensor_copy(at_sb[:], tp[0:D, 0:D])
                    nc.sync.dma_start(a2[D:2 * D, D:2 * D], at_sb[:])
            # T for both heads of the pair: one K=128 matmul per 512 cols
            for nf in range(2):
                ps = psB.tile([P, 512], F32, tag="psB", name=f"psT{b}_{pr}_{nf}",
                              space="PSUM")
                nc.tensor.matmul(ps[:], a2[:],
                                 qT[pr][:, nf * 512:(nf + 1) * 512],
                                 start=True, stop=True)
                nc.vector.tensor_copy(outT[pr][:, nf * 512:(nf + 1) * 512],
                                      ps[:])

        # ---- Phase D: y = out @ w_out ----
        for m in range(NM):
            yt = y_pool.tile([P, C], F32, tag="y", name=f"y{b}_{m}")
            for f in range(2):
                ps = psB.tile([P, 384], F32, tag="psB", name=f"psy{b}_{m}_{f}",
                              space="PSUM")
                for p in range(KC):
                    nc.tensor.matmul(
                        ps[:],
                        outT[p][:, m * P:(m + 1) * P],
                        wo[p][:, f * 384:(f + 1) * 384],
                        start=(p == 0), stop=(p == KC - 1))
                nc.vector.tensor_copy(yt[:, f * 384:(f + 1) * 384], ps[:])
            nc.sync.dma_start(y_d[b, m * P:(m + 1) * P, :], yt[:])


_BUILD_CACHE = {}


def build_program(mm_dt_name=MM_DT_NAME):
    if mm_dt_name in _BUILD_CACHE:
        return _BUILD_CACHE[mm_dt_name]
    mm_dt = F32R if mm_dt_name == "f32r" else F32
    nc = bacc.Bacc("TRN2", target_bir_lowering=False, debug=False,
                   num_devices=NCORES)
    x_d = nc.dram_tensor("x", [BS, N, C], F32, kind="ExternalInput").ap()
    wqkv_d = nc.dram_tensor("w_qkv", [C, 3 * C], F32, kind="ExternalInput").ap()
    wo_d = nc.dram_tensor("w_out", [C, C], F32, kind="ExternalInput").ap()
    y_d = nc.dram_tensor("y", [BS, N, C], F32, kind="ExternalOutput").ap()
    with tile.TileContext(nc) as tc:
        with ExitStack() as ctx:
            _emit(ctx, tc, mm_dt, x_d, wqkv_d, wo_d, y_d)
    nc.compile()
    _BUILD_CACHE[mm_dt_name] = nc
    return nc


def make_in_maps(x, w_qkv, w_out):
    x = np.ascontiguousarray(np.asarray(x, dtype=np.float32))
    w_qkv = np.ascontiguousarray(np.asarray(w_qkv, dtype=np.float32))
    w_out = np.ascontiguousarray(np.asarray(w_out, dtype=np.float32))
    return [
        {"x": x[i * BS:(i + 1) * BS], "w_qkv": w_qkv, "w_out": w_out}
        for i in range(NCORES)
    ]


def kernel(x, w_qkv, b_qkv=None, w_out=None, b_out=None, **_unused):
    nc = build_program()
    in_maps = make_in_maps(x, w_qkv, w_out)
    res = bass_utils.run_bass_kernel_spmd(nc, in_maps,
                                          core_ids=list(range(NCORES)))
    y = np.concatenate([res.results[i]["y"] for i in range(NCORES)], axis=0)
    return np.asarray(y, dtype=np.float32)
